# revision 9
# baseline (speedup 1.0000x reference)
# Trainium2 Bass kernel for nn_Net_4861902979707
#
# Computation (per sample, B = 4194304):
#   X [B, 3, 3] -> 3 pairwise Euclidean distances d = [d01, d02, d12]
#   h1 = elu(d @ W1.T + b1); h2 = elu(h1 @ W2.T + b2); y = h2 @ W3.T + b3
#
# Strategy: pure data parallel over 8 NeuronCores (batch split), sample-major
# layout on chip: tiles of [128 partitions, T samples each]. All compute on
# DVE/ACT/POOL (no matmuls -- the MLP is 3->2->2->1, tiny per sample).
# ELU is decomposed as elu(z)+1 = max(z,0) + exp(min(z,0)); the +1 shift is
# absorbed into the next layer's bias on the host (b' = b - W @ 1).
import numpy as np

B = 4194304
N_CORES = 8
B_CORE = B // N_CORES          # 524288
P = 128                        # partitions
T = 512                        # samples per partition per tile
TILE = P * T                   # 65536 samples per tile
N_TILES = B_CORE // TILE       # 8

# intermediate dtype: "bf16" (fast) or "fp32" (accurate)
COMPUTE_DT = "bf16"

_CACHE = {}


def _split_sync_waits(nc, mybir, limit=1):
    """This walrus build rejects instructions carrying more than ~1 sem wait
    ("Too many sync wait commands"). Hoist excess waits onto NoOp carrier
    instructions (same engine, immediately before) — engine program order
    preserves the blocking semantics."""
    n_split = 0
    for f in nc.m.functions:
        for b in f.blocks:
            lst = b.instructions
            out = []
            changed = False
            for inst in lst:
                si = inst.sync_info
                if si is not None and len(si.on_wait) > limit:
                    waits = list(si.on_wait)
                    extra, keep = waits[:-limit], waits[-limit:]
                    for wi, w in enumerate(extra):
                        nop = mybir.InstNoOp(
                            name=f"wsplit-{inst.name}-{wi}")
                        nop.engine = inst.engine
                        nop.sync_info = mybir.SyncInfo(
                            on_wait=[w], on_update=[])
                        out.append(nop)
                        n_split += 1
                    inst.sync_info = type(si)(
                        on_wait=keep, on_update=list(si.on_update))
                    changed = True
                out.append(inst)
            if changed:
                b.instructions = out
    return n_split


def _build(dt_name, reps=1, bench_small=False):
    import concourse.bass as bass
    import concourse.tile as tile
    import concourse.mybir as mybir

    f32 = mybir.dt.float32
    dt = mybir.dt.bfloat16 if dt_name == "bf16" else f32
    Alu = mybir.AluOpType
    Act = mybir.ActivationFunctionType

    nc = bass.Bass()
    BC = TILE if bench_small else B_CORE
    X = nc.dram_tensor("X", [BC, 9], f32, kind="ExternalInput")
    WB = nc.dram_tensor("WB", [17], f32, kind="ExternalInput")
    Y = nc.dram_tensor("Y", [BC, 1], f32, kind="ExternalOutput")

    # weight scalar indices in WB
    def iW1(k, j): return 3 * k + j
    def ib1(k): return 6 + k
    def iW2(m, j): return 8 + 2 * m + j
    def ib2(m): return 12 + m
    def iW3(j): return 14 + j
    IB3 = 16

    PAIRS = [(0, 1), (0, 2), (1, 2)]

    with tile.TileContext(nc) as tc:
        with (
            tc.tile_pool(name="singles", bufs=1) as singles,
            tc.tile_pool(name="xin", bufs=3) as xin,
            tc.tile_pool(name="work", bufs=2) as work,
            tc.tile_pool(name="mlp", bufs=2) as mlp,
            tc.tile_pool(name="yout", bufs=3) as yout,
        ):
            # broadcast the 17 weight scalars to all 128 partitions
            wb = singles.tile([P, 17], f32)
            wb_bcast = bass.AP(tensor=WB[:].tensor, offset=0,
                               ap=[[0, P], [1, 17]])
            nc.gpsimd.dma_start(out=wb[:], in_=wb_bcast)

            def ws(i):  # [P,1] scalar AP for weight i
                return wb[:, i:i + 1]

            # reps>1 wraps the whole body in a For_i loop (benchmarking only)
            _loop = tc.For_i(0, reps) if reps != 1 else None
            if _loop is not None:
                _loop.__enter__()

            for ti in range(N_TILES):
                src = 0 if bench_small else ti
                xr = X[src * TILE:(src + 1) * TILE, :].rearrange(
                    "(p s) d -> p s d", p=P)
                xt = xin.tile([P, T, 9], f32)
                nc.sync.dma_start(out=xt[:], in_=xr)

                # pairwise diffs -> coord-major planes [P, 9, T] (pair-major:
                # plane 3*pi+c = coord c of pair pi)
                diff = work.tile([P, 9, T], dt)
                for pi, (i, j) in enumerate(PAIRS):
                    nc.vector.tensor_sub(
                        diff[:, 3 * pi:3 * pi + 3, :].rearrange("p c t -> p t c"),
                        xt[:, :, 3 * i:3 * i + 3],
                        xt[:, :, 3 * j:3 * j + 3],
                    )

                # squares (contiguous, bf16 -> 2x mode on DVE)
                sq = work.tile([P, 9, T], dt)
                if dt_name == "bf16":
                    nc.vector.tensor_mul(sq[:], diff[:], diff[:])
                else:
                    nc.scalar.activation(sq[:], diff[:], Act.Square)

                # sum the 3 coords of each pair: planes {0,3,6}+{1,4,7}+{2,5,8}
                sqv = sq.rearrange("p (q c) t -> p q c t", q=3)
                tmp = work.tile([P, 3, T], dt)
                nc.gpsimd.tensor_add(tmp[:], sqv[:, :, 0, :], sqv[:, :, 1, :])
                qsum = work.tile([P, 3, T], dt)
                nc.gpsimd.tensor_add(qsum[:], tmp[:], sqv[:, :, 2, :])

                # distances
                dist = work.tile([P, 3, T], dt)
                nc.scalar.activation(dist[:], qsum[:], Act.Sqrt)
                d0, d1, d2 = dist[:, 0, :], dist[:, 1, :], dist[:, 2, :]

                # L1: z_k = W1[k,:] @ d + b1[k];  h_k = max(z,0) + exp(min(z,0))
                h1 = []
                for k in range(2):
                    a = mlp.tile([P, T], dt, tag=f"a1_{k}")
                    nc.vector.tensor_scalar(
                        out=a, in0=d0, scalar1=ws(iW1(k, 0)),
                        scalar2=ws(ib1(k)), op0=Alu.mult, op1=Alu.add)
                    u = mlp.tile([P, T], dt, tag=f"u1_{k}")
                    nc.vector.scalar_tensor_tensor(
                        out=u, in0=d1, scalar=ws(iW1(k, 1)), in1=a,
                        op0=Alu.mult, op1=Alu.add)
                    z = mlp.tile([P, T], dt, tag=f"z1_{k}")
                    nc.vector.scalar_tensor_tensor(
                        out=z, in0=d2, scalar=ws(iW1(k, 2)), in1=u,
                        op0=Alu.mult, op1=Alu.add)
                    m = mlp.tile([P, T], dt, tag=f"m1_{k}")
                    nc.vector.tensor_scalar_min(out=m, in0=z, scalar1=0.0)
                    e = mlp.tile([P, T], dt, tag=f"e1_{k}")
                    nc.scalar.activation(e, m, Act.Exp)
                    h = mlp.tile([P, T], dt, tag=f"h1_{k}")
                    nc.vector.scalar_tensor_tensor(
                        out=h, in0=z, scalar=0.0, in1=e,
                        op0=Alu.max, op1=Alu.add)
                    h1.append(h)

                # L2 (bias pre-adjusted on host: b2' = b2 - W2 @ 1)
                h2 = []
                for m_ in range(2):
                    a = mlp.tile([P, T], dt, tag=f"a2_{m_}")
                    nc.vector.tensor_scalar(
                        out=a, in0=h1[0], scalar1=ws(iW2(m_, 0)),
                        scalar2=ws(ib2(m_)), op0=Alu.mult, op1=Alu.add)
                    z = mlp.tile([P, T], dt, tag=f"z2_{m_}")
                    nc.vector.scalar_tensor_tensor(
                        out=z, in0=h1[1], scalar=ws(iW2(m_, 1)), in1=a,
                        op0=Alu.mult, op1=Alu.add)
                    mm = mlp.tile([P, T], dt, tag=f"m2_{m_}")
                    nc.vector.tensor_scalar_min(out=mm, in0=z, scalar1=0.0)
                    e = mlp.tile([P, T], dt, tag=f"e2_{m_}")
                    nc.scalar.activation(e, mm, Act.Exp)
                    h = mlp.tile([P, T], dt, tag=f"h2_{m_}")
                    nc.vector.scalar_tensor_tensor(
                        out=h, in0=z, scalar=0.0, in1=e,
                        op0=Alu.max, op1=Alu.add)
                    h2.append(h)

                # L3: y = W3 @ h2' + b3'  (b3' = b3 - W3 @ 1), fp32 out
                a3 = mlp.tile([P, T], dt, tag="a3")
                nc.vector.tensor_scalar(
                    out=a3, in0=h2[0], scalar1=ws(iW3(0)),
                    scalar2=ws(IB3), op0=Alu.mult, op1=Alu.add)
                yt = yout.tile([P, T], f32)
                nc.vector.scalar_tensor_tensor(
                    out=yt, in0=h2[1], scalar=ws(iW3(1)), in1=a3,
                    op0=Alu.mult, op1=Alu.add)

                yr = Y[src * TILE:(src + 1) * TILE, :].rearrange(
                    "(p s) d -> p (s d)", p=P)
                nc.sync.dma_start(out=yr, in_=yt[:])

            if _loop is not None:
                _loop.__exit__(None, None, None)

    _split_sync_waits(nc, mybir, limit=1)
    return nc


def _pack_weights(W1, b1, W2, b2, W3, b3):
    W1 = np.asarray(W1, np.float32); b1 = np.asarray(b1, np.float32)
    W2 = np.asarray(W2, np.float32); b2 = np.asarray(b2, np.float32)
    W3 = np.asarray(W3, np.float32); b3 = np.asarray(b3, np.float32)
    wb = np.empty(17, np.float32)
    wb[0:6] = W1.reshape(-1)
    wb[6:8] = b1
    wb[8:12] = W2.reshape(-1)
    wb[12:14] = b2 - W2.sum(axis=1)      # absorb elu(+1) shift
    wb[14:16] = W3.reshape(-1)
    wb[16] = b3[0] - W3.sum(axis=1)[0]
    return wb


LAST_RESULTS = None  # BassKernelResults of the most recent run (for test.py)


def kernel(X, W1, b1, W2, b2, W3, b3):
    from concourse.bass_utils import run_bass_kernel_spmd
    global LAST_RESULTS

    X = np.ascontiguousarray(np.asarray(X, np.float32).reshape(B, 9))
    wb = _pack_weights(W1, b1, W2, b2, W3, b3)

    key = (COMPUTE_DT, 1)
    if key not in _CACHE:
        _CACHE[key] = _build(COMPUTE_DT)
    nc = _CACHE[key]

    in_maps = [
        {"X": X[c * B_CORE:(c + 1) * B_CORE], "WB": wb}
        for c in range(N_CORES)
    ]
    res = run_bass_kernel_spmd(nc, in_maps, core_ids=list(range(N_CORES)))
    LAST_RESULTS = res
    out = np.concatenate([res.results[c]["Y"] for c in range(N_CORES)], axis=0)
    return out.reshape(B, 1)
